# revision 32
# baseline (speedup 1.0000x reference)
"""Polynomial features (degree 2) + linear layer, distributed over 8 TRN2 cores.

reference: A = [x, {x_i*x_j for i<=j}] (8384 coeffs); out = A @ W.T + b.

Device algorithm (per core, batch shard 4096, feature-on-partition layout):
  - pairs are enumerated by circular distance class s in 0..64:
      class s, lane p  ->  unordered pair {p, (p+s) % 128}
    (each unordered pair appears exactly once; s=64 lanes >=64 are dups
    with zeroed weights)
  - class products come from three sources, balancing DVE / PE+ACT:
      * class 0 (squares): ACT Square of x (SQ), with polar corrections
        folded into its weight block
      * DVE classes (48): bf16 tensor_mul of two rotated copies of x^T
        (rot d: row p = feature (p+d)%128), shipped from host
      * POLAR classes (16, anchor families 56 and 64): polarization
        x_a*x_b =
        ((x_a+x_b)^2 - x_a^2 - x_b^2)/2. The sum is a PE matmul with a
        0/1 permutation-sum stationary against un-rotated x; ACT
        evacuates PSUM with Square -> bf16 q_s; contraction uses W_s/2;
        the -x_a^2-x_b^2 corrections fold into the SQ block
  - batch tiles of 1024; each K-block contraction is two N=512 matmuls
    (PSUM bank limit) sharing one weight load, issued back-to-back so
    even/odd column-half pairs stream concurrently
  - 66 contraction K-blocks (linear + SQ + 64 classes) accumulate into
    PSUM halves (even -> partitions/cols 0:64, odd -> 64:128); a final
    identity matmul folds the odd half (ACT-copied to SBUF bf16) into
    the even accumulation; ACT adds bias in the single PSUM->SBUF copy;
    one plain DMA per tile writes out
  - TPB instructions have a single sync-wait slot, but Tile emits multiple
    waits on slot-recycling instructions; _split_multiwaits() post-processes
    the BIR, hoisting extra waits onto injected same-engine NOPs
"""

import numpy as np
import ml_dtypes

INPUT_DIM = 128
OUTPUT_DIM = 64
BATCH = 32768
N_CORES = 8
B_CORE = BATCH // N_CORES  # 4096
TILE_B = 1024
N_TILES = B_CORE // TILE_B  # 4
HALF = 512  # matmul moving free dim (PSUM bank limit)

# class partition (s = 1..64; class 0 is the SQ block)
DVE_GROUPS = (
    tuple(range(1, 9)),
    tuple(range(9, 17)),
    tuple(range(17, 25)),
    tuple(range(25, 33)),
    tuple(range(33, 41)),
    tuple(range(41, 47)),
)
HOST_CLASSES = (47, 48) + tuple(range(49, 57))
POLAR_CLASSES = tuple(range(57, 65))

ROT_SET = list(range(9)) + [16, 24, 32, 40, 48]
N_ROT = len(ROT_SET)
ROT_IDX = {d: i for i, d in enumerate(ROT_SET)}
N_SLOT = N_ROT  # host products ship in their own tensor (SWDGE stream)
N_HOST = len(HOST_CLASSES)
HOST_SLOT = {s: i for i, s in enumerate(HOST_CLASSES)}


def _class_ops():
    """(a, b) rotation pair per distance class s=0..64 with b - a = s."""
    ops = []
    for s in range(65):
        if s <= 8:
            a, b = 0, s
        else:
            k = (s - 1) // 8  # 1..7
            anchor = 8 * k + 8
            a, b = anchor - s, anchor
        ops.append((a, b))
    return ops


CLASS_OPS = _class_ops()
assert sorted(
    [0]
    + [s for g in DVE_GROUPS for s in g]
    + list(HOST_CLASSES)
    + list(POLAR_CLASSES)
) == list(range(65))
for g in DVE_GROUPS:
    for s in g:
        a, b = CLASS_OPS[s]
        assert a in ROT_SET and b in ROT_SET, (s, a, b)


def _build_device_weights(W, b):
    """Permute W [64, 8384] into the device K-block layout.

    Returns (w_packed [128, 66*64] bf16, s_packed [128, n_pol*128] bf16,
    bias f32). Block j=0 linear, j=1 SQ (class 0 + polar corrections),
    j=1+s class s (scaled 1/2 for polar classes). Class s row p -> pair
    {(p+a)%128, (p+a+s)%128}; duplicate lanes (s=64 second half) zeroed.
    """
    W = np.asarray(W, np.float32)
    n = INPUT_DIM
    pair_off = {}
    c = 0
    for i in range(n):
        for j in range(i, n):
            pair_off[(i, j)] = c
            c += 1
    assert c == 8256

    Wl = np.zeros((65, 128, OUTPUT_DIM), np.float32)
    seen = set()
    for s in range(65):
        a, _bb = CLASS_OPS[s]
        for p in range(128):
            u = (p + a) % 128
            v = (p + a + s) % 128
            i, j = (u, v) if u <= v else (v, u)
            if (i, j) in seen:
                continue  # duplicate lane (s=64 second half)
            seen.add((i, j))
            Wl[s, p] = W[:, 128 + pair_off[(i, j)]]
    assert len(seen) == 8256, len(seen)

    # polarization corrections: -1/2 sum_s (W_s scattered to x_a^2, x_b^2 lanes)
    C = np.zeros((128, OUTPUT_DIM), np.float32)
    for s in POLAR_CLASSES:
        a, bb = CLASS_OPS[s]
        for p in range(128):
            C[(p + a) % 128] += Wl[s, p]
            C[(p + bb) % 128] += Wl[s, p]
    C *= -0.5

    blocks = np.zeros((66, 128, OUTPUT_DIM), np.float32)
    blocks[0] = W[:, 0:128].T  # linear
    blocks[1] = Wl[0] + C  # SQ block
    for s in range(1, 65):
        blocks[1 + s] = Wl[s] * (0.5 if s in POLAR_CLASSES else 1.0)
    w_packed = np.ascontiguousarray(
        blocks.transpose(1, 0, 2).reshape(128, 66 * OUTPUT_DIM)
    ).astype(ml_dtypes.bfloat16)

    # 0/1 permutation-sum stationary matrices for polar classes:
    # out[p, n] = x[(p+a)%128, n] + x[(p+b)%128, n]
    n_pol = len(POLAR_CLASSES)
    S = np.zeros((n_pol, 128, 128), np.float32)
    for i, s in enumerate(POLAR_CLASSES):
        a, bb = CLASS_OPS[s]
        for p in range(128):
            S[i, (p + a) % 128, p] += 1.0
            S[i, (p + bb) % 128, p] += 1.0
    s_packed = np.ascontiguousarray(
        S.transpose(1, 0, 2).reshape(128, n_pol * 128)
    ).astype(ml_dtypes.bfloat16)

    return w_packed, s_packed, np.asarray(b, np.float32)


def _split_multiwaits(nc, mybir):
    """TPB instructions have one sync-wait slot; hoist extras onto NOPs."""
    import bass_rust

    n_split = 0
    for fn in nc.m.functions:
        for bb in fn.blocks:
            out = []
            changed = False
            for inst in bb.instructions:
                si = getattr(inst, "sync_info", None)
                if si is not None and si.on_wait and len(si.on_wait) > 1:
                    for w in si.on_wait[:-1]:
                        n_split += 1
                        nop = bass_rust.InstNoOp(
                            name=f"I-mw{n_split}",
                            engine=inst.engine,
                            ins=[],
                            outs=[],
                            sync_info=mybir.SyncInfo(on_wait=[w], on_update=[]),
                            bass_nofuse=True,
                        )
                        out.append(nop)
                    inst.sync_info = mybir.SyncInfo(
                        on_wait=[si.on_wait[-1]], on_update=si.on_update
                    )
                    changed = True
                out.append(inst)
            if changed:
                bb.instructions = out
    return n_split


def build(x, W, b):
    """Build the Bass graph and per-core input maps. Returns (nc, in_maps)."""
    import concourse.bass as bass
    import concourse.mybir as mybir
    from concourse import tile

    bf16 = mybir.dt.bfloat16
    f32 = mybir.dt.float32

    n_pol = len(POLAR_CLASSES)

    # ---- host preprocessing ----
    xT = np.ascontiguousarray(np.asarray(x, np.float32).T)  # [128, 32768] f32
    rotf = {d: np.roll(xT, -d, axis=0) for d in set(ROT_SET) | {48, 56}}
    xall = np.stack(
        [rotf[d].astype(ml_dtypes.bfloat16) for d in ROT_SET], axis=1
    )  # [128, N_SLOT, 32768]
    hall = np.stack(
        [
            (rotf[CLASS_OPS[s][0]] * rotf[CLASS_OPS[s][1]]).astype(
                ml_dtypes.bfloat16
            )
            for s in HOST_CLASSES
        ],
        axis=1,
    )  # [128, N_HOST, 32768]
    w_packed, s_packed, bias = _build_device_weights(W, b)

    # ---- device graph ----
    nc = bass.Bass()
    x_in = nc.declare_dram_parameter(
        "xall", [N_TILES, 128, N_SLOT, TILE_B], bf16, isOutput=False
    )
    h_in = nc.declare_dram_parameter(
        "hall", [N_TILES, 128, N_HOST, TILE_B], bf16, isOutput=False
    )
    w_in = nc.declare_dram_parameter("Wd", [128, 66 * 64], bf16, isOutput=False)
    s_in = nc.declare_dram_parameter(
        "Ssum", [128, n_pol * 128], bf16, isOutput=False
    )
    i_in = nc.declare_dram_parameter(
        "I64", [OUTPUT_DIM, OUTPUT_DIM], bf16, isOutput=False
    )
    b_in = nc.declare_dram_parameter("bias", [OUTPUT_DIM, 1], f32, isOutput=False)
    out_ext = nc.declare_dram_parameter(
        "outT", [OUTPUT_DIM, B_CORE], f32, isOutput=True
    )

    def rot_group_ap(xrt, classes):
        """[128, len(classes), TILE_B] APs (in0, in1)."""
        m = len(classes)
        us = [ROT_IDX[CLASS_OPS[s][0]] for s in classes]
        vs = [ROT_IDX[CLASS_OPS[s][1]] for s in classes]

        def mk(idx):
            if all(i == idx[0] for i in idx):
                return xrt[:, idx[0] : idx[0] + 1, :].to_broadcast(
                    [128, m, TILE_B]
                )
            d = idx[1] - idx[0]
            assert all(idx[j + 1] - idx[j] == d for j in range(m - 1)), idx
            return xrt[:, idx[0] :: d, :][:, 0:m, :]

        return mk(us), mk(vs)

    with tile.TileContext(nc) as tc:
        with (
            tc.tile_pool(name="consts", bufs=1) as consts,
            tc.tile_pool(name="xc", bufs=2) as xcp,
            tc.tile_pool(name="hp", bufs=2) as hpp,
            tc.tile_pool(name="prod", bufs=3) as prodp,
            tc.tile_pool(name="sq", bufs=2) as sqp,
            tc.tile_pool(name="q", bufs=4) as qp,
            tc.tile_pool(name="outp", bufs=2) as outp,
            tc.tile_pool(name="psum", bufs=2, space="PSUM") as psump,
            tc.tile_pool(name="psum_s", bufs=2, space="PSUM") as psump_s,
        ):
            xc_tiles = [None] * (N_TILES + 2)
            hp_tiles = [None] * (N_TILES + 2)

            def load_xc(t):
                if t >= N_TILES:
                    return
                xt = xcp.tile([128, N_SLOT, TILE_B], bf16, tag="xc", name="xc_t")
                nc.sync.dma_start(xt[:], x_in[t][:])
                xc_tiles[t] = xt

            def load_hp(t):
                if t >= N_TILES:
                    return
                # SWDGE (GpSimd) descriptor path: the host-product stream
                # cannot head-of-line-block the rotation stream on sync
                ht = hpp.tile([128, N_HOST, TILE_B], bf16, tag="hp", name="hp_t")
                nc.gpsimd.dma_start(ht[:], h_in[t][:])
                hp_tiles[t] = ht

            # DMA order: S (sums need it), first rotations (DVE), W, the
            # rest. The tiny bias/I64 transfers (~2us fixed cost each) go
            # after tile-1's load — they are first needed at ~30us. Host
            # products stream independently via SWDGE.
            s_sb = consts.tile([128, n_pol * 128], bf16)
            nc.sync.dma_start(s_sb[:], s_in[:])
            xt0 = xcp.tile([128, N_SLOT, TILE_B], bf16, tag="xc", name="xc_t")
            nc.sync.dma_start(xt0[:, 0:9, :], x_in[0][:, 0:9, :])
            w_sb = consts.tile([128, 66 * 64], bf16)
            nc.sync.dma_start(w_sb[:], w_in[:])
            nc.sync.dma_start(xt0[:, 9:N_SLOT, :], x_in[0][:, 9:N_SLOT, :])
            xc_tiles[0] = xt0
            load_hp(0)
            load_xc(1)
            load_hp(1)
            b_sb = consts.tile([OUTPUT_DIM, 1], f32)
            nc.sync.dma_start(b_sb[:], b_in[:])
            i64_sb = consts.tile([OUTPUT_DIM, OUTPUT_DIM], bf16)
            nc.sync.dma_start(i64_sb[:], i_in[:])

            for t in range(N_TILES):
                load_xc(t + 2)
                load_hp(t + 2)
                xrt = xc_tiles[t]
                hpt = hp_tiles[t]

                # SQ = x^2 (rot 0) on ACT
                sq_t = sqp.tile([128, TILE_B], bf16, tag="sq", name="sq_t")
                nc.scalar.activation(
                    sq_t[:],
                    xrt[:, 0, :],
                    mybir.ActivationFunctionType.Square,
                )

                # DVE product groups (shared padded tag -> one buffer ring)
                group_tiles = {}
                for classes in DVE_GROUPS:
                    m = len(classes)
                    p_t = prodp.tile(
                        [128, m, TILE_B],
                        bf16,
                        tag="prod",
                        name="p_t",
                        padded_shape=[128, 8, TILE_B],
                    )
                    in0, in1 = rot_group_ap(xrt, classes)
                    nc.vector.tensor_mul(p_t[:], in0, in1)
                    for j, s in enumerate(classes):
                        group_tiles[s] = (p_t, j)

                # polar sums on PE -> PSUM; ACT squares into bf16 q tiles
                q_tiles = {}
                next_sum = 0

                def issue_sum():
                    nonlocal next_sum
                    if next_sum >= n_pol:
                        return
                    i = next_sum
                    next_sum += 1
                    ps = psump_s.tile(
                        [128, 2, HALF], f32, tag="ps", name="ps_t"
                    )
                    for h in range(2):
                        nc.tensor.matmul(
                            ps[:, h, :],
                            s_sb[:, i * 128 : (i + 1) * 128],
                            xrt[:, 0, h * HALF : (h + 1) * HALF],
                            start=True,
                            stop=True,
                        )
                    q_t = qp.tile([128, TILE_B], bf16, tag="q", name="q_t")
                    nc.scalar.activation(
                        q_t[:],
                        ps[:],
                        mybir.ActivationFunctionType.Square,
                    )
                    q_tiles[POLAR_CLASSES[i]] = q_t

                issue_sum()
                issue_sum()

                # contraction: class-ordered, h-inner (adjacent same-weight
                # matmuls share an LDW; consecutive classes alternate parity
                # so even/odd column-half pairs stream concurrently)
                acc = psump.tile([128, 2, HALF], f32, name="acc")
                for h in range(2):
                    nc.tensor.matmul(
                        acc[0:64, h, :],
                        w_sb[:, 0:64],
                        xrt[:, 0, h * HALF : (h + 1) * HALF],
                        start=True,
                        stop=False,
                        tile_position=(0, 0),
                    )
                for h in range(2):
                    nc.tensor.matmul(
                        acc[0:64, h, :],
                        w_sb[:, 64:128],
                        sq_t[:, h * HALF : (h + 1) * HALF],
                        start=False,
                        stop=False,
                        tile_position=(0, 0),
                    )

                # polar and host classes interleaved between DVE groups in
                # 4-class runs: q tiles arrive from ACT at a matching pace,
                # host products arrive on the SWDGE stream, and the tail
                # after the last DVE product stays short
                pol = list(POLAR_CLASSES)
                host = list(HOST_CLASSES)
                fill = [
                    pol[0:4], pol[4:8],
                    host[0:4], host[4:8], host[8:10],
                ]
                order = list(DVE_GROUPS[0])
                for gi, g in enumerate(DVE_GROUPS[1:]):
                    order += fill[gi] + list(g)
                last_odd = max(s_ for s_ in order if s_ % 2 == 1)
                # verify parity alternation for pairing
                for a_, b_ in zip(order, order[1:]):
                    assert (a_ % 2) != (b_ % 2), (a_, b_)

                first_odd = [True, True]
                for ci, s in enumerate(order):
                    if s in POLAR_CLASSES:
                        while s not in q_tiles:
                            issue_sum()
                    half = s % 2
                    blk = 1 + s
                    for h in range(2):
                        if s in group_tiles:
                            p_t, j = group_tiles[s]
                            rhs = p_t[:, j, h * HALF : (h + 1) * HALF]
                        elif s in HOST_SLOT:
                            rhs = hpt[:, HOST_SLOT[s], h * HALF : (h + 1) * HALF]
                        else:
                            rhs = q_tiles[s][:, h * HALF : (h + 1) * HALF]
                        nc.tensor.matmul(
                            acc[64 * half : 64 * half + 64, h, :],
                            w_sb[:, blk * 64 : (blk + 1) * 64],
                            rhs,
                            start=(half == 1 and first_odd[h]),
                            stop=(half == 1 and s == last_odd),
                            tile_position=(0, 64 * half),
                        )
                        if half == 1:
                            first_odd[h] = False
                    if ci % 2 == 0:
                        issue_sum()

                # fold odd half into even accumulation via identity matmul
                o2_t = outp.tile(
                    [OUTPUT_DIM, TILE_B], bf16, tag="o2", name="o2_t"
                )
                nc.scalar.copy(o2_t[:], acc[64:128, :, :])
                for h in range(2):
                    nc.tensor.matmul(
                        acc[0:64, h, :],
                        i64_sb[:],
                        o2_t[:, h * HALF : (h + 1) * HALF],
                        start=False,
                        stop=True,
                        tile_position=(0, 0),
                    )
                o_t = outp.tile([OUTPUT_DIM, TILE_B], f32, tag="o", name="o_t")
                nc.scalar.activation(
                    o_t[:],
                    acc[0:64, :, :],
                    mybir.ActivationFunctionType.Identity,
                    bias=b_sb[:, 0:1],
                )
                bs = slice(t * TILE_B, (t + 1) * TILE_B)
                nc.sync.dma_start(out_ext[:, bs], o_t[:])

    _split_multiwaits(nc, mybir)

    # ---- per-core input maps ----
    i64 = np.eye(OUTPUT_DIM, dtype=np.float32).astype(ml_dtypes.bfloat16)
    in_maps = []
    for c in range(N_CORES):
        cs = xall[:, :, c * B_CORE : (c + 1) * B_CORE]  # [128, N_SLOT, 4096]
        xtiles = np.ascontiguousarray(
            cs.reshape(128, N_SLOT, N_TILES, TILE_B).transpose(2, 0, 1, 3)
        )  # [N_TILES, 128, N_SLOT, TILE_B]
        hs = hall[:, :, c * B_CORE : (c + 1) * B_CORE]
        htiles = np.ascontiguousarray(
            hs.reshape(128, N_HOST, N_TILES, TILE_B).transpose(2, 0, 1, 3)
        )  # [N_TILES, 128, N_HOST, TILE_B]
        in_maps.append(
            {
                "xall": xtiles,
                "hall": htiles,
                "Wd": w_packed,
                "Ssum": s_packed,
                "I64": i64,
                "bias": bias.reshape(OUTPUT_DIM, 1),
            }
        )
    return nc, in_maps


def kernel(x, W, b, indices_0, indices_1):
    from concourse.bass_utils import run_bass_kernel_spmd

    nc, in_maps = build(x, W, b)
    res = run_bass_kernel_spmd(nc, in_maps, list(range(N_CORES))).results
    out = np.concatenate([np.asarray(r["outT"], np.float32).T for r in res], axis=0)
    return out


# revision 33
# speedup vs baseline: 1.1963x; 1.1963x over previous
"""Polynomial features (degree 2) + linear layer, distributed over 8 TRN2 cores.

reference: A = [x, {x_i*x_j for i<=j}] (8384 coeffs); out = A @ W.T + b.

Device algorithm (per core, batch shard 4096, feature-on-partition layout):
  - pairs are enumerated by circular distance class s in 0..64:
      class s, lane p  ->  unordered pair {p, (p+s) % 128}
    (each unordered pair appears exactly once; s=64 lanes >=64 are dups
    with zeroed weights)
  - class products come from three sources, balancing DVE / PE+ACT:
      * class 0 (squares): ACT Square of x (SQ), with polar corrections
        folded into its weight block
      * DVE classes (48): bf16 tensor_mul of two rotated copies of x^T
        (rot d: row p = feature (p+d)%128), shipped from host
      * POLAR classes (16, anchor families 56 and 64): polarization
        x_a*x_b =
        ((x_a+x_b)^2 - x_a^2 - x_b^2)/2. The sum is a PE matmul with a
        0/1 permutation-sum stationary against un-rotated x; ACT
        evacuates PSUM with Square -> bf16 q_s; contraction uses W_s/2;
        the -x_a^2-x_b^2 corrections fold into the SQ block
  - batch tiles of 1024; each K-block contraction is two N=512 matmuls
    (PSUM bank limit) sharing one weight load, issued back-to-back so
    even/odd column-half pairs stream concurrently
  - 66 contraction K-blocks (linear + SQ + 64 classes) accumulate into
    PSUM halves (even -> partitions/cols 0:64, odd -> 64:128); a final
    identity matmul folds the odd half (ACT-copied to SBUF bf16) into
    the even accumulation; ACT adds bias in the single PSUM->SBUF copy;
    one plain DMA per tile writes out
  - TPB instructions have a single sync-wait slot, but Tile emits multiple
    waits on slot-recycling instructions; _split_multiwaits() post-processes
    the BIR, hoisting extra waits onto injected same-engine NOPs
"""

import numpy as np
import ml_dtypes

INPUT_DIM = 128
OUTPUT_DIM = 64
BATCH = 32768
N_CORES = 8
B_CORE = BATCH // N_CORES  # 4096
TILE_B = 1024
N_TILES = B_CORE // TILE_B  # 4
HALF = 512  # matmul moving free dim (PSUM bank limit)

# class partition (s = 1..64; class 0 is the SQ block)
DVE_GROUPS = (
    tuple(range(1, 9)),
    tuple(range(9, 17)),
    tuple(range(17, 25)),
    tuple(range(25, 33)),
    tuple(range(33, 41)),
    tuple(range(41, 49)),
    (49, 50),
)
HOST_CLASSES = ()
POLAR_CLASSES = tuple(range(51, 65))

ROT_SET = list(range(9)) + [16, 24, 32, 40, 48, 56]
N_ROT = len(ROT_SET)
ROT_IDX = {d: i for i, d in enumerate(ROT_SET)}
N_SLOT = N_ROT + len(HOST_CLASSES)
HOST_SLOT = {s: N_ROT + i for i, s in enumerate(HOST_CLASSES)}


def _class_ops():
    """(a, b) rotation pair per distance class s=0..64 with b - a = s."""
    ops = []
    for s in range(65):
        if s <= 8:
            a, b = 0, s
        else:
            k = (s - 1) // 8  # 1..7
            anchor = 8 * k + 8
            a, b = anchor - s, anchor
        ops.append((a, b))
    return ops


CLASS_OPS = _class_ops()
assert sorted(
    [0]
    + [s for g in DVE_GROUPS for s in g]
    + list(HOST_CLASSES)
    + list(POLAR_CLASSES)
) == list(range(65))
for g in DVE_GROUPS:
    for s in g:
        a, b = CLASS_OPS[s]
        assert a in ROT_SET and b in ROT_SET, (s, a, b)


def _build_device_weights(W, b):
    """Permute W [64, 8384] into the device K-block layout.

    Returns (w_packed [128, 66*64] bf16, s_packed [128, n_pol*128] bf16,
    bias f32). Block j=0 linear, j=1 SQ (class 0 + polar corrections),
    j=1+s class s (scaled 1/2 for polar classes). Class s row p -> pair
    {(p+a)%128, (p+a+s)%128}; duplicate lanes (s=64 second half) zeroed.
    """
    W = np.asarray(W, np.float32)
    n = INPUT_DIM
    pair_off = {}
    c = 0
    for i in range(n):
        for j in range(i, n):
            pair_off[(i, j)] = c
            c += 1
    assert c == 8256

    Wl = np.zeros((65, 128, OUTPUT_DIM), np.float32)
    seen = set()
    for s in range(65):
        a, _bb = CLASS_OPS[s]
        for p in range(128):
            u = (p + a) % 128
            v = (p + a + s) % 128
            i, j = (u, v) if u <= v else (v, u)
            if (i, j) in seen:
                continue  # duplicate lane (s=64 second half)
            seen.add((i, j))
            Wl[s, p] = W[:, 128 + pair_off[(i, j)]]
    assert len(seen) == 8256, len(seen)

    # polarization corrections: -1/2 sum_s (W_s scattered to x_a^2, x_b^2 lanes)
    C = np.zeros((128, OUTPUT_DIM), np.float32)
    for s in POLAR_CLASSES:
        a, bb = CLASS_OPS[s]
        for p in range(128):
            C[(p + a) % 128] += Wl[s, p]
            C[(p + bb) % 128] += Wl[s, p]
    C *= -0.5

    blocks = np.zeros((66, 128, OUTPUT_DIM), np.float32)
    blocks[0] = W[:, 0:128].T  # linear
    blocks[1] = Wl[0] + C  # SQ block
    for s in range(1, 65):
        blocks[1 + s] = Wl[s] * (0.5 if s in POLAR_CLASSES else 1.0)
    w_packed = np.ascontiguousarray(
        blocks.transpose(1, 0, 2).reshape(128, 66 * OUTPUT_DIM)
    ).astype(ml_dtypes.bfloat16)

    # 0/1 permutation-sum stationary matrices for polar classes:
    # out[p, n] = x[(p+a)%128, n] + x[(p+b)%128, n]
    n_pol = len(POLAR_CLASSES)
    S = np.zeros((n_pol, 128, 128), np.float32)
    for i, s in enumerate(POLAR_CLASSES):
        a, bb = CLASS_OPS[s]
        for p in range(128):
            S[i, (p + a) % 128, p] += 1.0
            S[i, (p + bb) % 128, p] += 1.0
    s_packed = np.ascontiguousarray(
        S.transpose(1, 0, 2).reshape(128, n_pol * 128)
    ).astype(ml_dtypes.bfloat16)

    return w_packed, s_packed, np.asarray(b, np.float32)


def _split_multiwaits(nc, mybir):
    """TPB instructions have one sync-wait slot; hoist extras onto NOPs."""
    import bass_rust

    n_split = 0
    for fn in nc.m.functions:
        for bb in fn.blocks:
            out = []
            changed = False
            for inst in bb.instructions:
                si = getattr(inst, "sync_info", None)
                if si is not None and si.on_wait and len(si.on_wait) > 1:
                    for w in si.on_wait[:-1]:
                        n_split += 1
                        nop = bass_rust.InstNoOp(
                            name=f"I-mw{n_split}",
                            engine=inst.engine,
                            ins=[],
                            outs=[],
                            sync_info=mybir.SyncInfo(on_wait=[w], on_update=[]),
                            bass_nofuse=True,
                        )
                        out.append(nop)
                    inst.sync_info = mybir.SyncInfo(
                        on_wait=[si.on_wait[-1]], on_update=si.on_update
                    )
                    changed = True
                out.append(inst)
            if changed:
                bb.instructions = out
    return n_split


def build(x, W, b):
    """Build the Bass graph and per-core input maps. Returns (nc, in_maps)."""
    import concourse.bass as bass
    import concourse.mybir as mybir
    from concourse import tile

    bf16 = mybir.dt.bfloat16
    f32 = mybir.dt.float32

    n_pol = len(POLAR_CLASSES)

    # ---- host preprocessing ----
    xT = np.ascontiguousarray(np.asarray(x, np.float32).T)  # [128, 32768] f32
    rotf = {d: np.roll(xT, -d, axis=0) for d in set(ROT_SET) | {48, 56}}
    slots = [rotf[d].astype(ml_dtypes.bfloat16) for d in ROT_SET]
    for s in HOST_CLASSES:
        a, bb = CLASS_OPS[s]
        slots.append((rotf[a] * rotf[bb]).astype(ml_dtypes.bfloat16))
    xall = np.stack(slots, axis=1)  # [128, N_SLOT, 32768]
    w_packed, s_packed, bias = _build_device_weights(W, b)

    # ---- device graph ----
    nc = bass.Bass()
    x_in = nc.declare_dram_parameter(
        "xall", [N_TILES, 128, N_SLOT, TILE_B], bf16, isOutput=False
    )
    w_in = nc.declare_dram_parameter("Wd", [128, 66 * 64], bf16, isOutput=False)
    s_in = nc.declare_dram_parameter(
        "Ssum", [128, n_pol * 128], bf16, isOutput=False
    )
    i_in = nc.declare_dram_parameter(
        "I64", [OUTPUT_DIM, OUTPUT_DIM], bf16, isOutput=False
    )
    b_in = nc.declare_dram_parameter("bias", [OUTPUT_DIM, 1], f32, isOutput=False)
    out_ext = nc.declare_dram_parameter(
        "outT", [OUTPUT_DIM, B_CORE], f32, isOutput=True
    )

    def rot_group_ap(xrt, classes):
        """[128, len(classes), TILE_B] APs (in0, in1)."""
        m = len(classes)
        us = [ROT_IDX[CLASS_OPS[s][0]] for s in classes]
        vs = [ROT_IDX[CLASS_OPS[s][1]] for s in classes]

        def mk(idx):
            if all(i == idx[0] for i in idx):
                return xrt[:, idx[0] : idx[0] + 1, :].to_broadcast(
                    [128, m, TILE_B]
                )
            d = idx[1] - idx[0]
            assert all(idx[j + 1] - idx[j] == d for j in range(m - 1)), idx
            return xrt[:, idx[0] :: d, :][:, 0:m, :]

        return mk(us), mk(vs)

    with tile.TileContext(nc) as tc:
        with (
            tc.tile_pool(name="consts", bufs=1) as consts,
            tc.tile_pool(name="xc", bufs=2) as xcp,
            tc.tile_pool(name="prod", bufs=3) as prodp,
            tc.tile_pool(name="sq", bufs=2) as sqp,
            tc.tile_pool(name="q", bufs=4) as qp,
            tc.tile_pool(name="outp", bufs=2) as outp,
            tc.tile_pool(name="psum", bufs=2, space="PSUM") as psump,
            tc.tile_pool(name="psum_s", bufs=2, space="PSUM") as psump_s,
        ):
            xc_tiles = [None] * (N_TILES + 2)

            def load_xc(t, first=False):
                if t >= N_TILES:
                    return
                xt = xcp.tile([128, N_SLOT, TILE_B], bf16, tag="xc", name="xc_t")
                if first:
                    # split so the first product op starts after rots 0..8
                    nc.sync.dma_start(xt[:, 0:9, :], x_in[t][:, 0:9, :])
                    nc.sync.dma_start(xt[:, 9:N_ROT, :], x_in[t][:, 9:N_ROT, :])
                else:
                    # rotations (DVE needs early) before host products
                    nc.sync.dma_start(
                        xt[:, 0:N_ROT, :], x_in[t][:, 0:N_ROT, :]
                    )
                if N_SLOT > N_ROT:
                    nc.sync.dma_start(
                        xt[:, N_ROT:N_SLOT, :], x_in[t][:, N_ROT:N_SLOT, :]
                    )
                xc_tiles[t] = xt

            # DMA order: S (sums need it), first rotations (DVE), W, the
            # rest. The tiny bias/I64 transfers (~2us fixed cost each) go
            # after tile-1's load — they are first needed at ~30us.
            s_sb = consts.tile([128, n_pol * 128], bf16)
            nc.sync.dma_start(s_sb[:], s_in[:])
            xt0 = xcp.tile([128, N_SLOT, TILE_B], bf16, tag="xc", name="xc_t")
            nc.sync.dma_start(xt0[:, 0:9, :], x_in[0][:, 0:9, :])
            w_sb = consts.tile([128, 66 * 64], bf16)
            nc.sync.dma_start(w_sb[:], w_in[:])
            nc.sync.dma_start(xt0[:, 9:N_SLOT, :], x_in[0][:, 9:N_SLOT, :])
            xc_tiles[0] = xt0
            load_xc(1)
            b_sb = consts.tile([OUTPUT_DIM, 1], f32)
            nc.sync.dma_start(b_sb[:], b_in[:])
            i64_sb = consts.tile([OUTPUT_DIM, OUTPUT_DIM], bf16)
            nc.sync.dma_start(i64_sb[:], i_in[:])

            for t in range(N_TILES):
                load_xc(t + 2)
                xrt = xc_tiles[t]

                # SQ = x^2 (rot 0) on ACT
                sq_t = sqp.tile([128, TILE_B], bf16, tag="sq", name="sq_t")
                nc.scalar.activation(
                    sq_t[:],
                    xrt[:, 0, :],
                    mybir.ActivationFunctionType.Square,
                )

                # DVE product groups
                group_tiles = {}
                for classes in DVE_GROUPS:
                    m = len(classes)
                    p_t = prodp.tile(
                        [128, m, TILE_B], bf16, tag=f"prod{m}", name="p_t"
                    )
                    in0, in1 = rot_group_ap(xrt, classes)
                    nc.vector.tensor_mul(p_t[:], in0, in1)
                    for j, s in enumerate(classes):
                        group_tiles[s] = (p_t, j)

                # polar sums on PE -> PSUM; ACT squares into bf16 q tiles
                q_tiles = {}
                next_sum = 0

                def issue_sum():
                    nonlocal next_sum
                    if next_sum >= n_pol:
                        return
                    i = next_sum
                    next_sum += 1
                    ps = psump_s.tile(
                        [128, 2, HALF], f32, tag="ps", name="ps_t"
                    )
                    for h in range(2):
                        nc.tensor.matmul(
                            ps[:, h, :],
                            s_sb[:, i * 128 : (i + 1) * 128],
                            xrt[:, 0, h * HALF : (h + 1) * HALF],
                            start=True,
                            stop=True,
                        )
                    q_t = qp.tile([128, TILE_B], bf16, tag="q", name="q_t")
                    nc.scalar.activation(
                        q_t[:],
                        ps[:],
                        mybir.ActivationFunctionType.Square,
                    )
                    q_tiles[POLAR_CLASSES[i]] = q_t

                issue_sum()
                issue_sum()

                # contraction: class-ordered, h-inner (adjacent same-weight
                # matmuls share an LDW; consecutive classes alternate parity
                # so even/odd column-half pairs stream concurrently)
                acc = psump.tile([128, 2, HALF], f32, name="acc")
                for h in range(2):
                    nc.tensor.matmul(
                        acc[0:64, h, :],
                        w_sb[:, 0:64],
                        xrt[:, 0, h * HALF : (h + 1) * HALF],
                        start=True,
                        stop=False,
                        tile_position=(0, 0),
                    )
                for h in range(2):
                    nc.tensor.matmul(
                        acc[0:64, h, :],
                        w_sb[:, 64:128],
                        sq_t[:, h * HALF : (h + 1) * HALF],
                        start=False,
                        stop=False,
                        tile_position=(0, 0),
                    )

                # polar classes interleaved between DVE groups in 4-class
                # runs: their q tiles arrive from ACT at a matching pace,
                # and the tail after the last DVE product stays short
                pol = list(POLAR_CLASSES)
                fill = [pol[0:4], pol[4:8], pol[8:12], pol[12:14], [], []]
                order = list(DVE_GROUPS[0])
                for gi, g in enumerate(DVE_GROUPS[1:]):
                    order += fill[gi] + list(g)
                last_odd = max(s_ for s_ in order if s_ % 2 == 1)
                # verify parity alternation for pairing
                for a_, b_ in zip(order, order[1:]):
                    assert (a_ % 2) != (b_ % 2), (a_, b_)

                first_odd = [True, True]
                for ci, s in enumerate(order):
                    if s in POLAR_CLASSES:
                        while s not in q_tiles:
                            issue_sum()
                    half = s % 2
                    blk = 1 + s
                    for h in range(2):
                        if s in group_tiles:
                            p_t, j = group_tiles[s]
                            rhs = p_t[:, j, h * HALF : (h + 1) * HALF]
                        elif s in HOST_SLOT:
                            rhs = xrt[:, HOST_SLOT[s], h * HALF : (h + 1) * HALF]
                        else:
                            rhs = q_tiles[s][:, h * HALF : (h + 1) * HALF]
                        nc.tensor.matmul(
                            acc[64 * half : 64 * half + 64, h, :],
                            w_sb[:, blk * 64 : (blk + 1) * 64],
                            rhs,
                            start=(half == 1 and first_odd[h]),
                            stop=(half == 1 and s == last_odd),
                            tile_position=(0, 64 * half),
                        )
                        if half == 1:
                            first_odd[h] = False
                    if ci % 2 == 0:
                        issue_sum()

                # fold odd half into even accumulation via identity matmul
                o2_t = outp.tile(
                    [OUTPUT_DIM, TILE_B], bf16, tag="o2", name="o2_t"
                )
                nc.scalar.copy(o2_t[:], acc[64:128, :, :])
                for h in range(2):
                    nc.tensor.matmul(
                        acc[0:64, h, :],
                        i64_sb[:],
                        o2_t[:, h * HALF : (h + 1) * HALF],
                        start=False,
                        stop=True,
                        tile_position=(0, 0),
                    )
                o_t = outp.tile([OUTPUT_DIM, TILE_B], f32, tag="o", name="o_t")
                nc.scalar.activation(
                    o_t[:],
                    acc[0:64, :, :],
                    mybir.ActivationFunctionType.Identity,
                    bias=b_sb[:, 0:1],
                )
                bs = slice(t * TILE_B, (t + 1) * TILE_B)
                nc.sync.dma_start(out_ext[:, bs], o_t[:])

    _split_multiwaits(nc, mybir)

    # ---- per-core input maps ----
    i64 = np.eye(OUTPUT_DIM, dtype=np.float32).astype(ml_dtypes.bfloat16)
    in_maps = []
    for c in range(N_CORES):
        cs = xall[:, :, c * B_CORE : (c + 1) * B_CORE]  # [128, N_SLOT, 4096]
        xtiles = np.ascontiguousarray(
            cs.reshape(128, N_SLOT, N_TILES, TILE_B).transpose(2, 0, 1, 3)
        )  # [N_TILES, 128, N_SLOT, TILE_B]
        in_maps.append(
            {
                "xall": xtiles,
                "Wd": w_packed,
                "Ssum": s_packed,
                "I64": i64,
                "bias": bias.reshape(OUTPUT_DIM, 1),
            }
        )
    return nc, in_maps


def kernel(x, W, b, indices_0, indices_1):
    from concourse.bass_utils import run_bass_kernel_spmd

    nc, in_maps = build(x, W, b)
    res = run_bass_kernel_spmd(nc, in_maps, list(range(N_CORES))).results
    out = np.concatenate([np.asarray(r["outT"], np.float32).T for r in res], axis=0)
    return out


# revision 35
# speedup vs baseline: 1.2439x; 1.0398x over previous
"""Polynomial features (degree 2) + linear layer, distributed over 8 TRN2 cores.

reference: A = [x, {x_i*x_j for i<=j}] (8384 coeffs); out = A @ W.T + b.

Device algorithm (per core, batch shard 4096, feature-on-partition layout):
  - pairs are enumerated by circular distance class s in 0..64:
      class s, lane p  ->  unordered pair {p, (p+s) % 128}
    (each unordered pair appears exactly once; s=64 lanes >=64 are dups
    with zeroed weights)
  - class products come from three sources, balancing DVE / PE+ACT:
      * class 0 (squares): ACT Square of x (SQ), with polar corrections
        folded into its weight block
      * DVE classes (50): bf16 tensor_mul of two rotated copies of x^T
        (rot d: row p = feature (p+d)%128), shipped from host
      * POLAR classes (14, s=51..64): polarization
        x_a*x_b =
        ((x_a+x_b)^2 - x_a^2 - x_b^2)/2. The sum is a PE matmul with a
        0/1 permutation-sum stationary against un-rotated x; ACT
        evacuates PSUM with Square -> bf16 q_s; contraction uses W_s/2;
        the -x_a^2-x_b^2 corrections fold into the SQ block
  - batch tiles of 1024; each K-block contraction is two N=512 matmuls
    (PSUM bank limit) sharing one weight load, issued back-to-back so
    even/odd column-half pairs stream concurrently
  - 66 contraction K-blocks (linear + SQ + 64 classes) accumulate into
    PSUM halves (even -> partitions/cols 0:64, odd -> 64:128); a final
    identity matmul folds the odd half (ACT-copied to SBUF bf16) into
    the even accumulation; ACT adds bias in the single PSUM->SBUF copy;
    one plain DMA per tile writes out
  - TPB instructions have a single sync-wait slot, but Tile emits multiple
    waits on slot-recycling instructions; _split_multiwaits() post-processes
    the BIR, hoisting extra waits onto injected same-engine NOPs
"""

import numpy as np
import ml_dtypes

INPUT_DIM = 128
OUTPUT_DIM = 64
BATCH = 32768
N_CORES = 8
B_CORE = BATCH // N_CORES  # 4096
TILE_B = 1024
N_TILES = B_CORE // TILE_B  # 4
HALF = 512  # matmul moving free dim (PSUM bank limit)

# class partition (s = 1..64; class 0 is the SQ block)
DVE_GROUPS = (
    tuple(range(1, 9)),
    tuple(range(9, 17)),
    tuple(range(17, 25)),
    tuple(range(25, 33)),
    tuple(range(33, 41)),
    tuple(range(41, 49)),
    (49, 50),
)
HOST_CLASSES = ()
POLAR_CLASSES = tuple(range(51, 65))

ROT_SET = list(range(9)) + [16, 24, 32, 40, 48, 56]
N_ROT = len(ROT_SET)
ROT_IDX = {d: i for i, d in enumerate(ROT_SET)}
N_SLOT = N_ROT + len(HOST_CLASSES)
HOST_SLOT = {s: N_ROT + i for i, s in enumerate(HOST_CLASSES)}


def _class_ops():
    """(a, b) rotation pair per distance class s=0..64 with b - a = s."""
    ops = []
    for s in range(65):
        if s <= 8:
            a, b = 0, s
        else:
            k = (s - 1) // 8  # 1..7
            anchor = 8 * k + 8
            a, b = anchor - s, anchor
        ops.append((a, b))
    return ops


CLASS_OPS = _class_ops()
assert sorted(
    [0]
    + [s for g in DVE_GROUPS for s in g]
    + list(HOST_CLASSES)
    + list(POLAR_CLASSES)
) == list(range(65))
for g in DVE_GROUPS:
    for s in g:
        a, b = CLASS_OPS[s]
        assert a in ROT_SET and b in ROT_SET, (s, a, b)


def _build_device_weights(W, b):
    """Permute W [64, 8384] into the device K-block layout.

    Returns (w_packed [128, 66*64] bf16, s_packed [128, n_pol*128] bf16,
    bias f32). Block j=0 linear, j=1 SQ (class 0 + polar corrections),
    j=1+s class s (scaled 1/2 for polar classes). Class s row p -> pair
    {(p+a)%128, (p+a+s)%128}; duplicate lanes (s=64 second half) zeroed.
    """
    W = np.asarray(W, np.float32)
    n = INPUT_DIM
    pair_off = {}
    c = 0
    for i in range(n):
        for j in range(i, n):
            pair_off[(i, j)] = c
            c += 1
    assert c == 8256

    Wl = np.zeros((65, 128, OUTPUT_DIM), np.float32)
    seen = set()
    for s in range(65):
        a, _bb = CLASS_OPS[s]
        for p in range(128):
            u = (p + a) % 128
            v = (p + a + s) % 128
            i, j = (u, v) if u <= v else (v, u)
            if (i, j) in seen:
                continue  # duplicate lane (s=64 second half)
            seen.add((i, j))
            Wl[s, p] = W[:, 128 + pair_off[(i, j)]]
    assert len(seen) == 8256, len(seen)

    # polarization corrections: -1/2 sum_s (W_s scattered to x_a^2, x_b^2 lanes)
    C = np.zeros((128, OUTPUT_DIM), np.float32)
    for s in POLAR_CLASSES:
        a, bb = CLASS_OPS[s]
        for p in range(128):
            C[(p + a) % 128] += Wl[s, p]
            C[(p + bb) % 128] += Wl[s, p]
    C *= -0.5

    blocks = np.zeros((66, 128, OUTPUT_DIM), np.float32)
    blocks[0] = W[:, 0:128].T  # linear
    blocks[1] = Wl[0] + C  # SQ block
    for s in range(1, 65):
        blocks[1 + s] = Wl[s] * (0.5 if s in POLAR_CLASSES else 1.0)
    w_packed = np.ascontiguousarray(
        blocks.transpose(1, 0, 2).reshape(128, 66 * OUTPUT_DIM)
    ).astype(ml_dtypes.bfloat16)

    # 0/1 permutation-sum stationary matrices for polar classes:
    # out[p, n] = x[(p+a)%128, n] + x[(p+b)%128, n]
    n_pol = len(POLAR_CLASSES)
    S = np.zeros((n_pol, 128, 128), np.float32)
    for i, s in enumerate(POLAR_CLASSES):
        a, bb = CLASS_OPS[s]
        for p in range(128):
            S[i, (p + a) % 128, p] += 1.0
            S[i, (p + bb) % 128, p] += 1.0
    s_packed = np.ascontiguousarray(
        S.transpose(1, 0, 2).reshape(128, n_pol * 128)
    ).astype(ml_dtypes.bfloat16)

    return w_packed, s_packed, np.asarray(b, np.float32)


def _split_multiwaits(nc, mybir):
    """TPB instructions have one sync-wait slot; hoist extras onto NOPs."""
    import bass_rust

    n_split = 0
    for fn in nc.m.functions:
        for bb in fn.blocks:
            out = []
            changed = False
            for inst in bb.instructions:
                si = getattr(inst, "sync_info", None)
                if si is not None and si.on_wait and len(si.on_wait) > 1:
                    for w in si.on_wait[:-1]:
                        n_split += 1
                        nop = bass_rust.InstNoOp(
                            name=f"I-mw{n_split}",
                            engine=inst.engine,
                            ins=[],
                            outs=[],
                            sync_info=mybir.SyncInfo(on_wait=[w], on_update=[]),
                            bass_nofuse=True,
                        )
                        out.append(nop)
                    inst.sync_info = mybir.SyncInfo(
                        on_wait=[si.on_wait[-1]], on_update=si.on_update
                    )
                    changed = True
                out.append(inst)
            if changed:
                bb.instructions = out
    return n_split


def build(x, W, b):
    """Build the Bass graph and per-core input maps. Returns (nc, in_maps)."""
    import concourse.bass as bass
    import concourse.mybir as mybir
    from concourse import tile

    bf16 = mybir.dt.bfloat16
    f32 = mybir.dt.float32

    n_pol = len(POLAR_CLASSES)

    # ---- host preprocessing ----
    xT = np.ascontiguousarray(np.asarray(x, np.float32).T)  # [128, 32768] f32
    rotf = {d: np.roll(xT, -d, axis=0) for d in set(ROT_SET) | {48, 56}}
    slots = [rotf[d].astype(ml_dtypes.bfloat16) for d in ROT_SET]
    for s in HOST_CLASSES:
        a, bb = CLASS_OPS[s]
        slots.append((rotf[a] * rotf[bb]).astype(ml_dtypes.bfloat16))
    xall = np.stack(slots, axis=1)  # [128, N_SLOT, 32768]
    w_packed, s_packed, bias = _build_device_weights(W, b)

    # ---- device graph ----
    nc = bass.Bass()
    x_in = nc.declare_dram_parameter(
        "xall", [N_TILES, 128, N_SLOT, TILE_B], bf16, isOutput=False
    )
    w_in = nc.declare_dram_parameter("Wd", [128, 66 * 64], bf16, isOutput=False)
    s_in = nc.declare_dram_parameter(
        "Ssum", [128, n_pol * 128], bf16, isOutput=False
    )
    i_in = nc.declare_dram_parameter(
        "I64", [OUTPUT_DIM, OUTPUT_DIM], bf16, isOutput=False
    )
    b_in = nc.declare_dram_parameter("bias", [OUTPUT_DIM, 1], f32, isOutput=False)
    out_ext = nc.declare_dram_parameter(
        "outT", [OUTPUT_DIM, B_CORE], f32, isOutput=True
    )

    def rot_group_ap(xrt, classes):
        """[128, len(classes), TILE_B] APs (in0, in1)."""
        m = len(classes)
        us = [ROT_IDX[CLASS_OPS[s][0]] for s in classes]
        vs = [ROT_IDX[CLASS_OPS[s][1]] for s in classes]

        def mk(idx):
            if all(i == idx[0] for i in idx):
                return xrt[:, idx[0] : idx[0] + 1, :].to_broadcast(
                    [128, m, TILE_B]
                )
            d = idx[1] - idx[0]
            assert all(idx[j + 1] - idx[j] == d for j in range(m - 1)), idx
            return xrt[:, idx[0] :: d, :][:, 0:m, :]

        return mk(us), mk(vs)

    with tile.TileContext(nc) as tc:
        with (
            tc.tile_pool(name="consts", bufs=1) as consts,
            tc.tile_pool(name="xc", bufs=2) as xcp,
            tc.tile_pool(name="prod", bufs=4) as prodp,
            tc.tile_pool(name="sq", bufs=2) as sqp,
            tc.tile_pool(name="q", bufs=5) as qp,
            tc.tile_pool(name="outp", bufs=2) as outp,
            tc.tile_pool(name="psum", bufs=2, space="PSUM") as psump,
            tc.tile_pool(name="psum_s", bufs=2, space="PSUM") as psump_s,
        ):
            xc_tiles = [None] * (N_TILES + 2)

            def load_xc(t, first=False):
                if t >= N_TILES:
                    return
                xt = xcp.tile([128, N_SLOT, TILE_B], bf16, tag="xc", name="xc_t")
                if first:
                    # split so the first product op starts after rots 0..8
                    nc.sync.dma_start(xt[:, 0:9, :], x_in[t][:, 0:9, :])
                    nc.sync.dma_start(xt[:, 9:N_ROT, :], x_in[t][:, 9:N_ROT, :])
                else:
                    # rotations (DVE needs early) before host products
                    nc.sync.dma_start(
                        xt[:, 0:N_ROT, :], x_in[t][:, 0:N_ROT, :]
                    )
                if N_SLOT > N_ROT:
                    nc.sync.dma_start(
                        xt[:, N_ROT:N_SLOT, :], x_in[t][:, N_ROT:N_SLOT, :]
                    )
                xc_tiles[t] = xt

            # DMA order: S (sums need it), first rotations (DVE), W, the
            # rest. The tiny bias/I64 transfers (~2us fixed cost each) go
            # after tile-1's load — they are first needed at ~30us.
            s_sb = consts.tile([128, n_pol * 128], bf16)
            nc.sync.dma_start(s_sb[:], s_in[:])
            xt0 = xcp.tile([128, N_SLOT, TILE_B], bf16, tag="xc", name="xc_t")
            nc.sync.dma_start(xt0[:, 0:9, :], x_in[0][:, 0:9, :])
            w_sb = consts.tile([128, 66 * 64], bf16)
            nc.sync.dma_start(w_sb[:], w_in[:])
            nc.sync.dma_start(xt0[:, 9:N_SLOT, :], x_in[0][:, 9:N_SLOT, :])
            xc_tiles[0] = xt0
            load_xc(1)
            b_sb = consts.tile([OUTPUT_DIM, 1], f32)
            nc.sync.dma_start(b_sb[:], b_in[:])
            i64_sb = consts.tile([OUTPUT_DIM, OUTPUT_DIM], bf16)
            nc.sync.dma_start(i64_sb[:], i_in[:])

            for t in range(N_TILES):
                load_xc(t + 2)
                xrt = xc_tiles[t]

                # SQ = x^2 (rot 0) on ACT
                sq_t = sqp.tile([128, TILE_B], bf16, tag="sq", name="sq_t")
                nc.scalar.activation(
                    sq_t[:],
                    xrt[:, 0, :],
                    mybir.ActivationFunctionType.Square,
                )

                # DVE product groups
                group_tiles = {}
                for classes in DVE_GROUPS:
                    m = len(classes)
                    p_t = prodp.tile(
                        [128, m, TILE_B], bf16, tag=f"prod{m}", name="p_t"
                    )
                    in0, in1 = rot_group_ap(xrt, classes)
                    nc.vector.tensor_mul(p_t[:], in0, in1)
                    for j, s in enumerate(classes):
                        group_tiles[s] = (p_t, j)

                # polar sums on PE -> PSUM; ACT squares into bf16 q tiles
                q_tiles = {}
                next_sum = 0

                def issue_sum():
                    nonlocal next_sum
                    if next_sum >= n_pol:
                        return
                    i = next_sum
                    next_sum += 1
                    ps = psump_s.tile(
                        [128, 2, HALF], f32, tag="ps", name="ps_t"
                    )
                    for h in range(2):
                        nc.tensor.matmul(
                            ps[:, h, :],
                            s_sb[:, i * 128 : (i + 1) * 128],
                            xrt[:, 0, h * HALF : (h + 1) * HALF],
                            start=True,
                            stop=True,
                        )
                    q_t = qp.tile([128, TILE_B], bf16, tag="q", name="q_t")
                    nc.scalar.activation(
                        q_t[:],
                        ps[:],
                        mybir.ActivationFunctionType.Square,
                    )
                    q_tiles[POLAR_CLASSES[i]] = q_t

                issue_sum()
                issue_sum()

                # contraction: class-ordered, h-inner (adjacent same-weight
                # matmuls share an LDW; consecutive classes alternate parity
                # so even/odd column-half pairs stream concurrently)
                acc = psump.tile([128, 2, HALF], f32, name="acc")
                for h in range(2):
                    nc.tensor.matmul(
                        acc[0:64, h, :],
                        w_sb[:, 0:64],
                        xrt[:, 0, h * HALF : (h + 1) * HALF],
                        start=True,
                        stop=False,
                        tile_position=(0, 0),
                    )
                for h in range(2):
                    nc.tensor.matmul(
                        acc[0:64, h, :],
                        w_sb[:, 64:128],
                        sq_t[:, h * HALF : (h + 1) * HALF],
                        start=False,
                        stop=False,
                        tile_position=(0, 0),
                    )

                # polar classes interleaved between DVE groups in 4-class
                # runs: their q tiles arrive from ACT at a matching pace,
                # and the tail after the last DVE product stays short
                pol = list(POLAR_CLASSES)
                fill = [pol[0:4], pol[4:8], pol[8:12], pol[12:14], [], []]
                order = list(DVE_GROUPS[0])
                for gi, g in enumerate(DVE_GROUPS[1:]):
                    order += fill[gi] + list(g)
                last_odd = max(s_ for s_ in order if s_ % 2 == 1)
                # verify parity alternation for pairing
                for a_, b_ in zip(order, order[1:]):
                    assert (a_ % 2) != (b_ % 2), (a_, b_)

                first_odd = [True, True]
                for ci, s in enumerate(order):
                    if s in POLAR_CLASSES:
                        while s not in q_tiles:
                            issue_sum()
                    half = s % 2
                    blk = 1 + s
                    for h in range(2):
                        if s in group_tiles:
                            p_t, j = group_tiles[s]
                            rhs = p_t[:, j, h * HALF : (h + 1) * HALF]
                        elif s in HOST_SLOT:
                            rhs = xrt[:, HOST_SLOT[s], h * HALF : (h + 1) * HALF]
                        else:
                            rhs = q_tiles[s][:, h * HALF : (h + 1) * HALF]
                        nc.tensor.matmul(
                            acc[64 * half : 64 * half + 64, h, :],
                            w_sb[:, blk * 64 : (blk + 1) * 64],
                            rhs,
                            start=(half == 1 and first_odd[h]),
                            stop=(half == 1 and s == last_odd),
                            tile_position=(0, 64 * half),
                        )
                        if half == 1:
                            first_odd[h] = False
                    if ci % 2 == 0:
                        issue_sum()

                # fold odd half into even accumulation via identity matmul
                o2_t = outp.tile(
                    [OUTPUT_DIM, TILE_B], bf16, tag="o2", name="o2_t"
                )
                nc.scalar.copy(o2_t[:], acc[64:128, :, :])
                for h in range(2):
                    nc.tensor.matmul(
                        acc[0:64, h, :],
                        i64_sb[:],
                        o2_t[:, h * HALF : (h + 1) * HALF],
                        start=False,
                        stop=True,
                        tile_position=(0, 0),
                    )
                o_t = outp.tile([OUTPUT_DIM, TILE_B], f32, tag="o", name="o_t")
                nc.scalar.activation(
                    o_t[:],
                    acc[0:64, :, :],
                    mybir.ActivationFunctionType.Identity,
                    bias=b_sb[:, 0:1],
                )
                bs = slice(t * TILE_B, (t + 1) * TILE_B)
                nc.sync.dma_start(out_ext[:, bs], o_t[:])

    _split_multiwaits(nc, mybir)

    # ---- per-core input maps ----
    i64 = np.eye(OUTPUT_DIM, dtype=np.float32).astype(ml_dtypes.bfloat16)
    in_maps = []
    for c in range(N_CORES):
        cs = xall[:, :, c * B_CORE : (c + 1) * B_CORE]  # [128, N_SLOT, 4096]
        xtiles = np.ascontiguousarray(
            cs.reshape(128, N_SLOT, N_TILES, TILE_B).transpose(2, 0, 1, 3)
        )  # [N_TILES, 128, N_SLOT, TILE_B]
        in_maps.append(
            {
                "xall": xtiles,
                "Wd": w_packed,
                "Ssum": s_packed,
                "I64": i64,
                "bias": bias.reshape(OUTPUT_DIM, 1),
            }
        )
    return nc, in_maps


def kernel(x, W, b, indices_0, indices_1):
    from concourse.bass_utils import run_bass_kernel_spmd

    nc, in_maps = build(x, W, b)
    res = run_bass_kernel_spmd(nc, in_maps, list(range(N_CORES))).results
    out = np.concatenate([np.asarray(r["outT"], np.float32).T for r in res], axis=0)
    return out
